# revision 25
# baseline (speedup 1.0000x reference)
"""MoE layer (8 experts, top-2) for 8 Trainium2 NeuronCores.

Strategy: expert-parallel with load rebalancing. Host computes the gate
(softmax + top-2) in numpy — this *is* the sharding decision — then
gathers each expert's tokens. Core e runs expert e's FFN
    y = (silu(x @ w1.T) * (x @ w3.T)) @ w2.T
on its tokens in bf16 (fp32 PSUM accumulation). Host combines with the
top-2 probabilities (scatter-add).

Load rebalancing: a uniform per-core column layout
    [ main C0 | flex1 V1 | flex2 V2 ]     (Ct = C0+V1+V2 columns)
lets overloaded experts spill overflow pieces into other cores' flex
slots. Main weights stay resident in SBUF; each flex slot's weight set
is a per-core input streamed chunk-by-chunk, with flex compute quanta
interleaved between main token tiles so the stream hides under main
compute. A greedy packer picks (C0, V1, V2) at the 16-column
granularity floor of max-core load; single-expert mode (V1=V2=0,
C0 = padded max load) is the fallback.

Device layout is feature-major ("transposed") throughout so no on-device
transposes are needed:
  xt   [D, Ct]            bf16  tokens for this core, d-major
  w1c  [KF, 128, KD, 128] bf16  w1.T chunked by output f-tile
  w3c  [KF, 128, KD, 128] bf16  w3.T chunked by output f-tile
  w2c  [KD, 128, KF, 128] bf16  w2.T chunked by output d-tile
  yt   [D, Ct]            bf16  core output, d-major
Weight chunking makes each output-tile's weights one contiguous DMA
(2 KB/partition), so compute starts after ~0.5 MB instead of 13 MB.
"""

import os
import sys
from contextlib import ExitStack

import numpy as np

sys.path.insert(0, "/opt/trn_rl_repo")

import ml_dtypes


def _ensure_axon_hooks():
    """bass_utils imports antenv.axon_hooks when tracing is requested (even
    via a stray BASS_TRACE env var); this container's antenv lacks that
    submodule. Provide a no-op fallback so the import never crashes."""
    import types

    if "antenv.axon_hooks" in sys.modules:
        return
    try:
        from antenv import axon_hooks  # noqa: F401

        return
    except ImportError:
        pass
    mod = types.ModuleType("antenv.axon_hooks")
    _state = {"hook": None}
    mod.get_axon_ntff_profile_hook = lambda: _state["hook"]
    mod.set_axon_ntff_profile_hook = lambda h: _state.__setitem__("hook", h)
    sys.modules["antenv.axon_hooks"] = mod
    try:
        import antenv

        antenv.axon_hooks = mod
    except ImportError:
        pass


_ensure_axon_hooks()

# ---- problem constants (hardcoded; kernel.py must be self-contained) ----
B, T, D, F, E, TOP_K = 8, 2048, 1024, 2048, 8, 2
N = B * T
NCORES = 8
KD = D // 128   # 8  contraction chunks over D
KF = F // 128   # 16 contraction chunks over F

_compiled = {}


def _tok_tiles(C):
    """Token-tile widths: 512s then one remainder (multiple of 16)."""
    sizes = [512] * (C // 512)
    if C % 512:
        sizes.append(C % 512)
    return sizes


def _pad16(v):
    return -(-v // 16) * 16


def _try_pack(loads, C0, V1, V2, tries=400):
    """Cut per-expert overflow (load - C0) into pieces, one piece per flex
    slot. Greedy (largest remainder into the current slot) over shuffled
    slot orders — the first try is slots-descending; retries explore
    orders where small slots trim large remainders early, which the pure
    descending greedy misses. Returns [(expert, core, slot_id, count)]
    or None."""
    import random as _random

    base = [[l - C0, e] for e, l in enumerate(loads) if l > C0]
    slots = []
    for c in range(NCORES):
        if V1:
            slots.append((V1, c, 1))
        if V2:
            slots.append((V2, c, 2))
    if sum(r for r, _ in base) > sum(s[0] for s in slots):
        return None
    rng = _random.Random(0)
    order0 = sorted(range(len(slots)), key=lambda i: -slots[i][0])
    for t in range(tries):
        order = order0 if t == 0 else rng.sample(range(len(slots)), len(slots))
        rem = [[r, e] for r, e in base]
        pieces = []
        for i in order:
            if not rem:
                break
            rem.sort(reverse=True)
            size, c, sid = slots[i]
            take = min(rem[0][0], size)
            pieces.append((rem[0][1], c, sid, take))
            rem[0][0] -= take
            if rem[0][0] == 0:
                rem.pop(0)
        if not rem:
            return pieces
    return None


def _mmcost(v):
    """Effective ns per matmul instruction at free-dim width v: the PE
    can't beat ~25ns/instr (LDWEIGHTS floor), so narrow-slot width is
    nearly free up to ~53 columns."""
    return max(25.0, 0.4167 * v + 2.7)


def _plan(loads):
    """Pick (C0, V1, V2, pieces) minimizing estimated PE time
    (160ns/main-column + per-slot instruction cost), not just Ct.
    Fallback: single-expert mode."""
    fallback = _pad16(max(loads))
    floor_ct = _pad16(-(-sum(loads) // NCORES))
    best = None
    for Ct in range(floor_ct, min(fallback, floor_ct + 64), 16):
        for V1 in range(16, 513, 16):
            for V2 in range(0, V1 + 1, 16):
                C0 = Ct - V1 - V2
                if C0 < 2048:
                    continue
                score = 160.0 * C0 + 384.0 * (
                    _mmcost(V1) + (_mmcost(V2) if V2 else 0.0)
                )
                if best is not None and score >= best[0]:
                    continue
                pieces = _try_pack(loads, C0, V1, V2)
                if pieces is not None:
                    best = (score, C0, V1, V2, pieces)
    if best is None:
        return fallback, 0, 0, []
    return best[1], best[2], best[3], best[4]


def _build_bass(C0: int, V1: int, V2: int, act: str = "Silu"):
    """Build the SPMD Bass program for layout [C0 | V1 | V2]."""
    import concourse.bacc as bacc
    import concourse.tile as tile
    from concourse import mybir

    act_fn = getattr(mybir.ActivationFunctionType, act)

    bf16 = mybir.dt.bfloat16
    f32 = mybir.dt.float32

    Ct = C0 + V1 + V2
    flex = [s for s in ((1, V1), (2, V2)) if s[1] > 0]

    nc = bacc.Bacc(
        "TRN2", target_bir_lowering=False, debug=False, num_devices=NCORES
    )
    xt = nc.declare_dram_parameter("xt", [D, Ct], bf16, isOutput=False)
    w1c = nc.declare_dram_parameter("w1c", [KF, 128, KD, 128], bf16, isOutput=False)
    w3c = nc.declare_dram_parameter("w3c", [KF, 128, KD, 128], bf16, isOutput=False)
    w2c = nc.declare_dram_parameter("w2c", [KD, 128, KF, 128], bf16, isOutput=False)
    fw = {}
    for sid, _v in flex:
        fw[sid] = tuple(
            nc.declare_dram_parameter(f"f{sid}_{nm}", shp, bf16, isOutput=False)
            for nm, shp in (
                ("w1c", [KF, 128, KD, 128]),
                ("w3c", [KF, 128, KD, 128]),
                ("w2c", [KD, 128, KF, 128]),
            )
        )
    yt = nc.declare_dram_parameter("yt", [D, Ct], bf16, isOutput=True)

    xt_r = xt.rearrange("(k p) n -> p k n", p=128)   # [128, KD, Ct]
    yt_r = yt.rearrange("(k p) n -> p k n", p=128)   # [128, KD, Ct]

    sizes = _tok_tiles(C0)
    starts = np.cumsum([0] + sizes[:-1]).tolist()
    T_tiles = len(sizes)
    fbase = {1: C0, 2: C0 + V1}
    fwidth = {1: V1, 2: V2}

    with ExitStack() as ctx:
        tc = ctx.enter_context(tile.TileContext(nc))
        wpool = ctx.enter_context(tc.tile_pool(name="w", bufs=1))
        xpool = ctx.enter_context(tc.tile_pool(name="x", bufs=3))
        hpool = ctx.enter_context(tc.tile_pool(name="h", bufs=2))
        spool = ctx.enter_context(tc.tile_pool(name="s", bufs=3))
        opool = ctx.enter_context(tc.tile_pool(name="o", bufs=4))
        psh = ctx.enter_context(tc.tile_pool(name="psh", bufs=2, space="PSUM"))
        psy = ctx.enter_context(tc.tile_pool(name="psy", bufs=2, space="PSUM"))
        if flex:
            fxpool = ctx.enter_context(tc.tile_pool(name="fx", bufs=1))
            fhpool = ctx.enter_context(tc.tile_pool(name="fh", bufs=1))
            fwpool = ctx.enter_context(tc.tile_pool(name="fw", bufs=5))
            fw2pool = ctx.enter_context(tc.tile_pool(name="fw2", bufs=4))
        warmp = ctx.enter_context(tc.tile_pool(name="warmp", bufs=1))
        warmps = ctx.enter_context(tc.tile_pool(name="warmps", bufs=1, space="PSUM"))

        # Tuned PE warm-up: the HAM clock gate needs ~5.3µs of PE-busy
        # time at 1.2 GHz before switching to 2.4 GHz, and the first real
        # matmul can't start before its DMA lands (~14µs). 12 scratch
        # matmuls (~5.1µs at mid clock) starting as soon as the engines
        # boot (~9µs) bank the ramp so real work starts at full speed.
        # The memset runs on DVE (GpSimd boots slower).
        wsrc = warmp.tile([128, 512], bf16, name="wsrc")
        nc.vector.memset(wsrc[:], 0.0)
        wdst = warmps.tile([128, 512], f32, name="wdst")
        for _ in range(10):
            nc.tensor.matmul(wdst[:], wsrc[:, 0:128], wsrc[:], start=True, stop=True)

        # Weights resident in SBUF for the whole kernel, one tile per
        # output chunk. DMA order: first f-chunk + first x tile first so
        # the PE starts as soon as the engines boot (~9µs); the rest
        # streams behind it. No PE warm-up: the clock ramp penalty
        # (~3µs at 1.2 GHz) is cheaper than idling the PE behind a
        # warm-up chain (~7µs).
        w1f = [
            wpool.tile([128, KD, 128], bf16, tag=f"w1f{f}", name=f"w1f{f}")
            for f in range(KF)
        ]
        w3f = [
            wpool.tile([128, KD, 128], bf16, tag=f"w3f{f}", name=f"w3f{f}")
            for f in range(KF)
        ]
        w2d = [
            wpool.tile([128, KF, 128], bf16, tag=f"w2d{d}", name=f"w2d{d}")
            for d in range(KD)
        ]

        def load_x(ts, tw):
            xs = [
                xpool.tile([128, tw], bf16, tag=f"xs{k}", name=f"xs{k}")
                for k in range(KD)
            ]
            for k in range(KD):
                nc.sync.dma_start(xs[k][:], xt_r[:, k, ts])
            return xs

        # DMA order matters twice here. First: the f=0 weights and tile-0
        # x chunks gate the first matmul, so they're split into per-k
        # strips interleaved so the earliest-consumed bytes land first
        # across ~16 parallel queues (single-queue delivery of a 256 KB
        # chunk costs ~9µs). Second: the PE consumes w1f/w3f chunks at
        # ~74 GB/s during tile 0's h-phase, so the f=1.. weight stream
        # must queue before anything not needed until later — a late
        # chunk shows up as a PE stall plus a clock re-ramp.
        # NOTE: splitting these first chunks into per-k strips to start the
        # PE earlier was tried and is a net loss: strips break the 2 KB/
        # partition contiguity of the chunk layout into 256 B bursts, DMA
        # delivery slows, and the PE stutters through f=1..6 with the
        # clock stuck at mid-speed. Whole chunks start the PE at ~14µs
        # with a clean ramp and a stream that stays ahead.
        # f=0 chunks split in halves (1 KB/partition bursts — still
        # DMA-efficient, unlike per-k 256 B strips) so two queues each
        # deliver the first weights ~2.5µs sooner.
        nc.sync.dma_start(w1f[0][:, 0:4, :], w1c[0, :, 0:4, :])
        nc.sync.dma_start(w1f[0][:, 4:8, :], w1c[0, :, 4:8, :])
        nc.sync.dma_start(w3f[0][:, 0:4, :], w3c[0, :, 0:4, :])
        nc.sync.dma_start(w3f[0][:, 4:8, :], w3c[0, :, 4:8, :])
        xs0 = load_x(slice(0, sizes[0]), sizes[0])
        for f in range(1, KF):
            nc.sync.dma_start(w1f[f][:], w1c[f])
            nc.sync.dma_start(w3f[f][:], w3c[f])
        for d in range(KD):
            nc.sync.dma_start(w2d[d][:], w2c[d])

        # flex token columns: small, loaded once, resident; not needed
        # until tile 1's quanta, so they queue behind the weight stream.
        fxs, hsF = {}, {}
        for sid, v in flex:
            fxs[sid] = [
                fxpool.tile(
                    [128, v], bf16, tag=f"fx{sid}_{k}", name=f"fx{sid}_{k}"
                )
                for k in range(KD)
            ]
            for k in range(KD):
                nc.sync.dma_start(
                    fxs[sid][k][:], xt_r[:, k, fbase[sid] : fbase[sid] + v]
                )
            hsF[sid] = fhpool.tile(
                [128, KF, v], bf16, tag=f"hsF{sid}", name=f"hsF{sid}"
            )

        # ---- flex compute quanta (interleaved between main tiles) ----
        def flex_h(sid, f):
            v = fwidth[sid]
            w1t = fwpool.tile([128, KD, 128], bf16, tag="fw1")
            nc.sync.dma_start(w1t[:], fw[sid][0][f])
            w3t = fwpool.tile([128, KD, 128], bf16, tag="fw3")
            nc.sync.dma_start(w3t[:], fw[sid][1][f])
            ph1 = psh.tile([128, v], f32, tag="ph1")
            ph3 = psh.tile([128, v], f32, tag="ph3")
            for k in range(KD):
                nc.tensor.matmul(
                    ph1[:], w1t[:, k, :], fxs[sid][k][:],
                    start=(k == 0), stop=(k == KD - 1),
                )
            for k in range(KD):
                nc.tensor.matmul(
                    ph3[:], w3t[:, k, :], fxs[sid][k][:],
                    start=(k == 0), stop=(k == KD - 1),
                )
            sil = spool.tile([128, v], f32, tag="sil")
            nc.scalar.activation(sil[:], ph1[:], act_fn)
            nc.vector.tensor_mul(hsF[sid][:, f, :], sil[:], ph3[:])

        def flex_y(sid, d):
            v = fwidth[sid]
            w2t = fw2pool.tile([128, KF, 128], bf16, tag="fw2")
            nc.sync.dma_start(w2t[:], fw[sid][2][d])
            py = psy.tile([128, v], f32, tag="py")
            for f in range(KF):
                nc.tensor.matmul(
                    py[:], w2t[:, f, :], hsF[sid][:, f, :],
                    start=(f == 0), stop=(f == KF - 1),
                )
            yo = opool.tile([128, v], bf16, tag="yo")
            nc.vector.tensor_copy(yo[:], py[:])
            nc.sync.dma_start(yt_r[:, d, fbase[sid] : fbase[sid] + v], yo[:])

        # schedule: tile 0 stays quanta-free (its window feeds the main
        # weight stream); h-quanta spread over tiles 1..~T/2, y-quanta
        # over the remaining tiles. Each quantum eats 512 KB of streamed
        # weights in <1µs of PE time while the stream delivers at
        # ~1.4µs/chunk, so quanta are woven INSIDE the main f/d loops
        # (one per ~2 main chunks) — bunched at a tile boundary they
        # outrun the prefetch pool and stall the PE (plus clock re-ramp).
        hq = []
        for f in range(KF):
            for sid, _v in flex:
                hq.append(("h", sid, f))
        yq = []
        for d in range(KD):
            for sid, _v in flex:
                yq.append(("y", sid, d))
        sched = [[] for _ in range(T_tiles)]
        leftover = []
        h_lo = min(1, T_tiles - 1)
        h_hi = max(T_tiles // 2 + 1, h_lo + 1)   # h-quanta on tiles [h_lo, h_hi)
        n_h_tiles = h_hi - h_lo
        n_y_tiles = T_tiles - h_hi               # y-quanta on tiles [h_hi, T)
        if n_h_tiles <= 0 or n_h_tiles * 8 < len(hq):
            leftover += hq
        else:
            for i, q in enumerate(hq):
                sched[h_lo + (i * n_h_tiles) // len(hq)].append(q)
        if n_y_tiles <= 0 or n_y_tiles * 16 < len(yq):
            leftover += yq
        else:
            for i, q in enumerate(yq):
                sched[h_hi + (i * n_y_tiles) // len(yq)].append(q)

        def emit(q):
            if q[0] == "h":
                flex_h(q[1], q[2])
            else:
                flex_y(q[1], q[2])

        # ---- main tiles ----
        for t, (t0, tw) in enumerate(zip(starts, sizes)):
            ts = slice(t0, t0 + tw)
            xs = xs0 if t == 0 else load_x(ts, tw)
            qlist = sched[t]
            qi = 0

            hs = hpool.tile([128, KF, tw], bf16, tag="hs")
            for f in range(KF):
                ph1 = psh.tile([128, tw], f32, tag="ph1")
                ph3 = psh.tile([128, tw], f32, tag="ph3")
                for k in range(KD):
                    nc.tensor.matmul(
                        ph1[:], w1f[f][:, k, :], xs[k][:],
                        start=(k == 0), stop=(k == KD - 1),
                    )
                for k in range(KD):
                    nc.tensor.matmul(
                        ph3[:], w3f[f][:, k, :], xs[k][:],
                        start=(k == 0), stop=(k == KD - 1),
                    )
                sil = spool.tile([128, tw], f32, tag="sil")
                nc.scalar.activation(sil[:], ph1[:], act_fn)
                nc.vector.tensor_mul(hs[:, f, :], sil[:], ph3[:])
                if f % 2 == 1 and qi < len(qlist):
                    emit(qlist[qi])
                    qi += 1

            for d in range(KD):
                py = psy.tile([128, tw], f32, tag="py")
                for f in range(KF):
                    nc.tensor.matmul(
                        py[:], w2d[d][:, f, :], hs[:, f, :],
                        start=(f == 0), stop=(f == KF - 1),
                    )
                yo = opool.tile([128, tw], bf16, tag="yo")
                nc.vector.tensor_copy(yo[:], py[:])
                nc.sync.dma_start(yt_r[:, d, ts], yo[:])
                if qi < len(qlist):
                    emit(qlist[qi])
                    qi += 1

            while qi < len(qlist):
                emit(qlist[qi])
                qi += 1

        for q in leftover:
            emit(q)

    nc.compile()
    return nc


def _route(xf: np.ndarray, gate_w: np.ndarray):
    """Numpy replica of the reference gate: softmax + top-2 + renorm."""
    logits = xf @ gate_w.T  # [N, E] f32
    m = logits.max(axis=-1, keepdims=True)
    p = np.exp(logits - m, dtype=np.float32)
    p /= p.sum(axis=-1, keepdims=True)
    i1 = np.argmax(p, axis=-1)
    ar = np.arange(N)
    pm = p.copy()
    pm[ar, i1] = -1.0
    i2 = np.argmax(pm, axis=-1)
    p1 = p[ar, i1]
    p2 = p[ar, i2]
    s = p1 + p2
    return i1, i2, (p1 / s).astype(np.float32), (p2 / s).astype(np.float32)


def _chunk_w13(wt):
    """[D, F] -> [KF, 128, KD, 128]: out[f, p, k, j] = wt[k*128+p, f*128+j]."""
    return np.ascontiguousarray(
        wt.reshape(KD, 128, KF, 128).transpose(2, 1, 0, 3)
    )


def _chunk_w2(wt):
    """[F, D] -> [KD, 128, KF, 128]: out[d, p, k, j] = wt[k*128+p, d*128+j]."""
    return np.ascontiguousarray(
        wt.reshape(KF, 128, KD, 128).transpose(2, 1, 0, 3)
    )


last_results = None  # BassKernelResults of the most recent run (for test harness)


def kernel(x, gate_w, w1, w2, w3):
    from concourse.bass_utils import run_bass_kernel_spmd

    xf = np.ascontiguousarray(np.asarray(x, dtype=np.float32).reshape(N, D))
    gate_w = np.asarray(gate_w, dtype=np.float32)
    i1, i2, c1, c2 = _route(xf, gate_w)

    # per-expert token lists (a token appears at most once per expert)
    idxs, combs = [], []
    for e in range(E):
        a = np.where(i1 == e)[0]
        b = np.where(i2 == e)[0]
        idxs.append(np.concatenate([a, b]))
        combs.append(np.concatenate([c1[a], c2[b]]))
    loads = [len(ix) for ix in idxs]

    C0, V1, V2, flex_pieces = _plan(loads)
    Ct = C0 + V1 + V2
    if (C0, V1, V2) not in _compiled:
        _compiled[(C0, V1, V2)] = _build_bass(C0, V1, V2)
    nc = _compiled[(C0, V1, V2)]

    bf = ml_dtypes.bfloat16
    w1b = np.asarray(w1, dtype=np.float32)
    w2b = np.asarray(w2, dtype=np.float32)
    w3b = np.asarray(w3, dtype=np.float32)
    w1ce = [_chunk_w13(w1b[e].T.astype(bf)) for e in range(E)]
    w3ce = [_chunk_w13(w3b[e].T.astype(bf)) for e in range(E)]
    w2ce = [_chunk_w2(w2b[e].T.astype(bf)) for e in range(E)]

    # piece bookkeeping: per core, (expert, col_start, token_slice)
    fbase = {1: C0, 2: C0 + V1}
    cursor = [0] * E
    core_pieces = [[] for _ in range(NCORES)]
    slot_expert = [[0, 0] for _ in range(NCORES)]  # experts for slots 1, 2
    for e in range(E):
        take = min(loads[e], C0)
        core_pieces[e].append((e, 0, 0, take))
        cursor[e] = take
    for e, c, sid, cnt in flex_pieces:
        core_pieces[c].append((e, fbase[sid], cursor[e], cnt))
        slot_expert[c][sid - 1] = e
        cursor[e] += cnt

    in_maps = []
    for c in range(NCORES):
        xg = np.zeros((Ct, D), dtype=bf)
        for e, col, tok0, cnt in core_pieces[c]:
            ix = idxs[e][tok0 : tok0 + cnt]
            xg[col : col + cnt] = xf[ix].astype(bf)
        m = {
            "xt": np.ascontiguousarray(xg.T),
            "w1c": w1ce[c],
            "w3c": w3ce[c],
            "w2c": w2ce[c],
        }
        if V1:
            e1 = slot_expert[c][0]
            m["f1_w1c"], m["f1_w3c"], m["f1_w2c"] = w1ce[e1], w3ce[e1], w2ce[e1]
        if V2:
            e2 = slot_expert[c][1]
            m["f2_w1c"], m["f2_w3c"], m["f2_w2c"] = w1ce[e2], w3ce[e2], w2ce[e2]
        in_maps.append(m)

    trace = os.environ.get("BASS_KERNEL_TRACE", "") not in ("", "0")
    res = run_bass_kernel_spmd(
        nc, in_maps, core_ids=list(range(NCORES)), trace=trace
    )
    global last_results
    last_results = res

    out = np.zeros((N, D), dtype=np.float32)
    for c in range(NCORES):
        yT = np.asarray(res.results[c]["yt"], dtype=np.float32)  # [D, Ct]
        for e, col, tok0, cnt in core_pieces[c]:
            ix = idxs[e][tok0 : tok0 + cnt]
            cb = combs[e][tok0 : tok0 + cnt]
            out[ix] += cb[:, None] * yT.T[col : col + cnt]
    return out.reshape(B, T, D)


# revision 33
# speedup vs baseline: 1.0001x; 1.0001x over previous
"""MoE layer (8 experts, top-2) for 8 Trainium2 NeuronCores.

Strategy: expert-parallel with load rebalancing. Host computes the gate
(softmax + top-2) in numpy — this *is* the sharding decision — then
gathers each expert's tokens. Core e runs expert e's FFN
    y = (silu(x @ w1.T) * (x @ w3.T)) @ w2.T
on its tokens in bf16 (fp32 PSUM accumulation). Host combines with the
top-2 probabilities (scatter-add).

Load rebalancing: a uniform per-core column layout
    [ main C0 | flex1 V1 | flex2 V2 ]     (Ct = C0+V1+V2 columns)
lets overloaded experts spill overflow pieces into other cores' flex
slots. Main weights stay resident in SBUF; each flex slot's weight set
is a per-core input streamed chunk-by-chunk, with flex compute quanta
interleaved between main token tiles so the stream hides under main
compute. A greedy packer picks (C0, V1, V2) at the 16-column
granularity floor of max-core load; single-expert mode (V1=V2=0,
C0 = padded max load) is the fallback.

Device layout is feature-major ("transposed") throughout so no on-device
transposes are needed:
  xt   [D, Ct]            bf16  tokens for this core, d-major
  w1c  [KF, 128, KD, 128] bf16  w1.T chunked by output f-tile
  w3c  [KF, 128, KD, 128] bf16  w3.T chunked by output f-tile
  w2c  [KD, 128, KF, 128] bf16  w2.T chunked by output d-tile
  yt   [D, Ct]            bf16  core output, d-major
Weight chunking makes each output-tile's weights one contiguous DMA
(2 KB/partition), so compute starts after ~0.5 MB instead of 13 MB.
"""

import os
import sys
from contextlib import ExitStack

import numpy as np

sys.path.insert(0, "/opt/trn_rl_repo")

import ml_dtypes


def _ensure_axon_hooks():
    """bass_utils imports antenv.axon_hooks when tracing is requested (even
    via a stray BASS_TRACE env var); this container's antenv lacks that
    submodule. Provide a no-op fallback so the import never crashes."""
    import types

    if "antenv.axon_hooks" in sys.modules:
        return
    try:
        from antenv import axon_hooks  # noqa: F401

        return
    except ImportError:
        pass
    mod = types.ModuleType("antenv.axon_hooks")
    _state = {"hook": None}
    mod.get_axon_ntff_profile_hook = lambda: _state["hook"]
    mod.set_axon_ntff_profile_hook = lambda h: _state.__setitem__("hook", h)
    sys.modules["antenv.axon_hooks"] = mod
    try:
        import antenv

        antenv.axon_hooks = mod
    except ImportError:
        pass


_ensure_axon_hooks()

# ---- problem constants (hardcoded; kernel.py must be self-contained) ----
B, T, D, F, E, TOP_K = 8, 2048, 1024, 2048, 8, 2
N = B * T
NCORES = 8
KD = D // 128   # 8  contraction chunks over D
KF = F // 128   # 16 contraction chunks over F

_compiled = {}


def _tok_tiles(C):
    """Token-tile widths: 512s then one remainder (multiple of 16)."""
    sizes = [512] * (C // 512)
    if C % 512:
        sizes.append(C % 512)
    return sizes


def _pad16(v):
    return -(-v // 16) * 16


def _try_pack(loads, C0, V1, V2, tries=400):
    """Cut per-expert overflow (load - C0) into pieces, one piece per flex
    slot. Greedy (largest remainder into the current slot) over shuffled
    slot orders — the first try is slots-descending; retries explore
    orders where small slots trim large remainders early, which the pure
    descending greedy misses. Returns [(expert, core, slot_id, count)]
    or None."""
    import random as _random

    base = [[l - C0, e] for e, l in enumerate(loads) if l > C0]
    slots = []
    for c in range(NCORES):
        if V1:
            slots.append((V1, c, 1))
        if V2:
            slots.append((V2, c, 2))
    if sum(r for r, _ in base) > sum(s[0] for s in slots):
        return None
    rng = _random.Random(0)
    order0 = sorted(range(len(slots)), key=lambda i: -slots[i][0])
    for t in range(tries):
        order = order0 if t == 0 else rng.sample(range(len(slots)), len(slots))
        rem = [[r, e] for r, e in base]
        pieces = []
        for i in order:
            if not rem:
                break
            rem.sort(reverse=True)
            size, c, sid = slots[i]
            take = min(rem[0][0], size)
            pieces.append((rem[0][1], c, sid, take))
            rem[0][0] -= take
            if rem[0][0] == 0:
                rem.pop(0)
        if not rem:
            return pieces
    return None


def _mmcost(v):
    """Effective ns per matmul instruction at free-dim width v: the PE
    can't beat ~25ns/instr (LDWEIGHTS floor), so narrow-slot width is
    nearly free up to ~53 columns."""
    return max(25.0, 0.4167 * v + 2.7)


def _plan(loads):
    """Pick (C0, V1, V2, pieces) minimizing estimated PE time
    (160ns/main-column + per-slot instruction cost), not just Ct.
    Fallback: single-expert mode."""
    fallback = _pad16(max(loads))
    floor_ct = _pad16(-(-sum(loads) // NCORES))
    best = None
    for Ct in range(floor_ct, min(fallback, floor_ct + 64), 16):
        for V1 in range(16, 513, 16):
            for V2 in range(0, V1 + 1, 16):
                C0 = Ct - V1 - V2
                if C0 < 2048:
                    continue
                score = 160.0 * C0 + 384.0 * (
                    _mmcost(V1) + (_mmcost(V2) if V2 else 0.0)
                )
                if best is not None and score >= best[0]:
                    continue
                pieces = _try_pack(loads, C0, V1, V2)
                if pieces is not None:
                    best = (score, C0, V1, V2, pieces)
    if best is None:
        return fallback, 0, 0, []
    return best[1], best[2], best[3], best[4]


def _build_bass(C0: int, V1: int, V2: int, act: str = "Silu"):
    """Build the SPMD Bass program for layout [C0 | V1 | V2]."""
    import concourse.bacc as bacc
    import concourse.tile as tile
    from concourse import mybir

    act_fn = getattr(mybir.ActivationFunctionType, act)

    bf16 = mybir.dt.bfloat16
    f32 = mybir.dt.float32

    Ct = C0 + V1 + V2
    flex = [s for s in ((1, V1), (2, V2)) if s[1] > 0]

    nc = bacc.Bacc(
        "TRN2", target_bir_lowering=False, debug=False, num_devices=NCORES
    )
    xt = nc.declare_dram_parameter("xt", [D, Ct], bf16, isOutput=False)
    w1c = nc.declare_dram_parameter("w1c", [KF, 128, KD, 128], bf16, isOutput=False)
    w3c = nc.declare_dram_parameter("w3c", [KF, 128, KD, 128], bf16, isOutput=False)
    w2c = nc.declare_dram_parameter("w2c", [KD, 128, KF, 128], bf16, isOutput=False)
    fw = {}
    for sid, _v in flex:
        fw[sid] = tuple(
            nc.declare_dram_parameter(f"f{sid}_{nm}", shp, bf16, isOutput=False)
            for nm, shp in (
                ("w1c", [KF, 128, KD, 128]),
                ("w3c", [KF, 128, KD, 128]),
                ("w2c", [KD, 128, KF, 128]),
            )
        )
    yt = nc.declare_dram_parameter("yt", [D, Ct], bf16, isOutput=True)

    xt_r = xt.rearrange("(k p) n -> p k n", p=128)   # [128, KD, Ct]
    yt_r = yt.rearrange("(k p) n -> p k n", p=128)   # [128, KD, Ct]

    sizes = _tok_tiles(C0)
    starts = np.cumsum([0] + sizes[:-1]).tolist()
    T_tiles = len(sizes)
    fbase = {1: C0, 2: C0 + V1}
    fwidth = {1: V1, 2: V2}

    with ExitStack() as ctx:
        tc = ctx.enter_context(tile.TileContext(nc))
        wpool = ctx.enter_context(tc.tile_pool(name="w", bufs=1))
        x0pool = ctx.enter_context(tc.tile_pool(name="x0", bufs=1))
        xpool = ctx.enter_context(tc.tile_pool(name="x", bufs=2))
        hpool = ctx.enter_context(tc.tile_pool(name="h", bufs=2))
        spool = ctx.enter_context(tc.tile_pool(name="s", bufs=3))
        opool = ctx.enter_context(tc.tile_pool(name="o", bufs=4))
        psh = ctx.enter_context(tc.tile_pool(name="psh", bufs=2, space="PSUM"))
        psy = ctx.enter_context(tc.tile_pool(name="psy", bufs=2, space="PSUM"))
        if flex:
            fxpool = ctx.enter_context(tc.tile_pool(name="fx", bufs=1))
            fhpool = ctx.enter_context(tc.tile_pool(name="fh", bufs=1))
            fwpool = ctx.enter_context(tc.tile_pool(name="fw", bufs=5))
            fw2pool = ctx.enter_context(tc.tile_pool(name="fw2", bufs=4))
        warmp = ctx.enter_context(tc.tile_pool(name="warmp", bufs=1))
        warmps = ctx.enter_context(tc.tile_pool(name="warmps", bufs=1, space="PSUM"))

        # Tuned PE warm-up: the HAM clock gate needs ~5.3µs of PE-busy
        # time at 1.2 GHz before switching to 2.4 GHz, and the first real
        # matmul can't start before its DMA lands (~14µs). 12 scratch
        # matmuls (~5.1µs at mid clock) starting as soon as the engines
        # boot (~9µs) bank the ramp so real work starts at full speed.
        # The memset runs on DVE (GpSimd boots slower).
        wsrc = warmp.tile([128, 512], bf16, name="wsrc")
        nc.vector.memset(wsrc[:], 0.0)
        wdst = warmps.tile([128, 512], f32, name="wdst")
        for _ in range(12):
            nc.tensor.matmul(wdst[:], wsrc[:, 0:128], wsrc[:], start=True, stop=True)

        # Weights resident in SBUF for the whole kernel, one tile per
        # output chunk. DMA order: first f-chunk + first x tile first so
        # the PE starts as soon as the engines boot (~9µs); the rest
        # streams behind it. No PE warm-up: the clock ramp penalty
        # (~3µs at 1.2 GHz) is cheaper than idling the PE behind a
        # warm-up chain (~7µs).
        w1f = [
            wpool.tile([128, KD, 128], bf16, tag=f"w1f{f}", name=f"w1f{f}")
            for f in range(KF)
        ]
        w3f = [
            wpool.tile([128, KD, 128], bf16, tag=f"w3f{f}", name=f"w3f{f}")
            for f in range(KF)
        ]
        w2d = [
            wpool.tile([128, KF, 128], bf16, tag=f"w2d{d}", name=f"w2d{d}")
            for d in range(KD)
        ]

        def load_x(ts, tw):
            # One 3-D DMA for all KD chunks: a single DMA→PE semaphore
            # instead of 8 (~53ns sem-wait each on first touch). Only for
            # tiles with a full tile of DMA lead time — tile 0 stays
            # split per-k across 8 queues for delivery latency.
            xs = xpool.tile([128, KD, tw], bf16, tag="xsA", name="xsA")
            nc.sync.dma_start(xs[:], xt_r[:, :, ts])
            return lambda k: xs[:, k, :]

        # DMA order matters twice here. First: the f=0 weights and tile-0
        # x chunks gate the first matmul, so they're split into per-k
        # strips interleaved so the earliest-consumed bytes land first
        # across ~16 parallel queues (single-queue delivery of a 256 KB
        # chunk costs ~9µs). Second: the PE consumes w1f/w3f chunks at
        # ~74 GB/s during tile 0's h-phase, so the f=1.. weight stream
        # must queue before anything not needed until later — a late
        # chunk shows up as a PE stall plus a clock re-ramp.
        # NOTE: splitting these first chunks into per-k strips to start the
        # PE earlier was tried and is a net loss: strips break the 2 KB/
        # partition contiguity of the chunk layout into 256 B bursts, DMA
        # delivery slows, and the PE stutters through f=1..6 with the
        # clock stuck at mid-speed. Whole chunks start the PE at ~14µs
        # with a clean ramp and a stream that stays ahead.
        nc.sync.dma_start(w1f[0][:], w1c[0])
        nc.sync.dma_start(w3f[0][:], w3c[0])
        xs0t = [
            x0pool.tile([128, sizes[0]], bf16, tag=f"xs0_{k}", name=f"xs0_{k}")
            for k in range(KD)
        ]
        for k in range(KD):
            nc.sync.dma_start(xs0t[k][:], xt_r[:, k, 0 : sizes[0]])
        xs0 = lambda k: xs0t[k][:]
        for f in range(1, KF):
            nc.sync.dma_start(w1f[f][:], w1c[f])
            nc.sync.dma_start(w3f[f][:], w3c[f])
        for d in range(KD):
            nc.sync.dma_start(w2d[d][:], w2c[d])

        # flex token columns: small, loaded once, resident; not needed
        # until tile 1's quanta, so they queue behind the weight stream.
        fxs, hsF = {}, {}
        for sid, v in flex:
            fxs[sid] = fxpool.tile(
                [128, KD, v], bf16, tag=f"fx{sid}", name=f"fx{sid}"
            )
            nc.sync.dma_start(
                fxs[sid][:], xt_r[:, :, fbase[sid] : fbase[sid] + v]
            )
            hsF[sid] = fhpool.tile(
                [128, KF, v], bf16, tag=f"hsF{sid}", name=f"hsF{sid}"
            )

        # ---- flex compute quanta (interleaved between main tiles) ----
        def flex_h(sid, f):
            v = fwidth[sid]
            w1t = fwpool.tile([128, KD, 128], bf16, tag="fw1")
            nc.sync.dma_start(w1t[:], fw[sid][0][f])
            w3t = fwpool.tile([128, KD, 128], bf16, tag="fw3")
            nc.sync.dma_start(w3t[:], fw[sid][1][f])
            ph1 = psh.tile([128, v], f32, tag="ph1")
            ph3 = psh.tile([128, v], f32, tag="ph3")
            for k in range(KD):
                nc.tensor.matmul(
                    ph1[:], w1t[:, k, :], fxs[sid][:, k, :],
                    start=(k == 0), stop=(k == KD - 1),
                )
            for k in range(KD):
                nc.tensor.matmul(
                    ph3[:], w3t[:, k, :], fxs[sid][:, k, :],
                    start=(k == 0), stop=(k == KD - 1),
                )
            sil = spool.tile([128, v], f32, tag="sil")
            nc.scalar.activation(sil[:], ph1[:], act_fn)
            nc.vector.tensor_mul(hsF[sid][:, f, :], sil[:], ph3[:])

        def flex_y(sid, d):
            v = fwidth[sid]
            w2t = fw2pool.tile([128, KF, 128], bf16, tag="fw2")
            nc.sync.dma_start(w2t[:], fw[sid][2][d])
            py = psy.tile([128, v], f32, tag="py")
            for f in range(KF):
                nc.tensor.matmul(
                    py[:], w2t[:, f, :], hsF[sid][:, f, :],
                    start=(f == 0), stop=(f == KF - 1),
                )
            yo = opool.tile([128, v], bf16, tag="yo")
            nc.vector.tensor_copy(yo[:], py[:])
            nc.sync.dma_start(yt_r[:, d, fbase[sid] : fbase[sid] + v], yo[:])

        # schedule: tile 0 stays quanta-free (its window feeds the main
        # weight stream); h-quanta spread over tiles 1..~T/2, y-quanta
        # over the remaining tiles. Each quantum eats 512 KB of streamed
        # weights in <1µs of PE time while the stream delivers at
        # ~1.4µs/chunk, so quanta are woven INSIDE the main f/d loops
        # (one per ~2 main chunks) — bunched at a tile boundary they
        # outrun the prefetch pool and stall the PE (plus clock re-ramp).
        hq = []
        for f in range(KF):
            for sid, _v in flex:
                hq.append(("h", sid, f))
        yq = []
        for d in range(KD):
            for sid, _v in flex:
                yq.append(("y", sid, d))
        sched = [[] for _ in range(T_tiles)]
        leftover = []
        h_lo = min(1, T_tiles - 1)
        h_hi = max(T_tiles // 2 + 1, h_lo + 1)   # h-quanta on tiles [h_lo, h_hi)
        n_h_tiles = h_hi - h_lo
        n_y_tiles = T_tiles - h_hi               # y-quanta on tiles [h_hi, T)
        if n_h_tiles <= 0 or n_h_tiles * 8 < len(hq):
            leftover += hq
        else:
            for i, q in enumerate(hq):
                sched[h_lo + (i * n_h_tiles) // len(hq)].append(q)
        if n_y_tiles <= 0 or n_y_tiles * 16 < len(yq):
            leftover += yq
        else:
            for i, q in enumerate(yq):
                sched[h_hi + (i * n_y_tiles) // len(yq)].append(q)

        def emit(q):
            if q[0] == "h":
                flex_h(q[1], q[2])
            else:
                flex_y(q[1], q[2])

        # ---- main tiles ----
        for t, (t0, tw) in enumerate(zip(starts, sizes)):
            ts = slice(t0, t0 + tw)
            xs = xs0 if t == 0 else load_x(ts, tw)
            qlist = sched[t]
            qi = 0

            hs = hpool.tile([128, KF, tw], bf16, tag="hs")
            for f in range(KF):
                ph1 = psh.tile([128, tw], f32, tag="ph1")
                ph3 = psh.tile([128, tw], f32, tag="ph3")
                for k in range(KD):
                    nc.tensor.matmul(
                        ph1[:], w1f[f][:, k, :], xs(k),
                        start=(k == 0), stop=(k == KD - 1),
                    )
                for k in range(KD):
                    nc.tensor.matmul(
                        ph3[:], w3f[f][:, k, :], xs(k),
                        start=(k == 0), stop=(k == KD - 1),
                    )
                sil = spool.tile([128, tw], f32, tag="sil")
                nc.scalar.activation(sil[:], ph1[:], act_fn)
                nc.vector.tensor_mul(hs[:, f, :], sil[:], ph3[:])
                if f % 2 == 1 and qi < len(qlist):
                    emit(qlist[qi])
                    qi += 1

            for d in range(KD):
                py = psy.tile([128, tw], f32, tag="py")
                for f in range(KF):
                    nc.tensor.matmul(
                        py[:], w2d[d][:, f, :], hs[:, f, :],
                        start=(f == 0), stop=(f == KF - 1),
                    )
                yo = opool.tile([128, tw], bf16, tag="yo")
                nc.vector.tensor_copy(yo[:], py[:])
                nc.sync.dma_start(yt_r[:, d, ts], yo[:])
                if qi < len(qlist):
                    emit(qlist[qi])
                    qi += 1

            while qi < len(qlist):
                emit(qlist[qi])
                qi += 1

        for q in leftover:
            emit(q)

    nc.compile()
    return nc


def _route(xf: np.ndarray, gate_w: np.ndarray):
    """Numpy replica of the reference gate: softmax + top-2 + renorm."""
    logits = xf @ gate_w.T  # [N, E] f32
    m = logits.max(axis=-1, keepdims=True)
    p = np.exp(logits - m, dtype=np.float32)
    p /= p.sum(axis=-1, keepdims=True)
    i1 = np.argmax(p, axis=-1)
    ar = np.arange(N)
    pm = p.copy()
    pm[ar, i1] = -1.0
    i2 = np.argmax(pm, axis=-1)
    p1 = p[ar, i1]
    p2 = p[ar, i2]
    s = p1 + p2
    return i1, i2, (p1 / s).astype(np.float32), (p2 / s).astype(np.float32)


def _chunk_w13(wt):
    """[D, F] -> [KF, 128, KD, 128]: out[f, p, k, j] = wt[k*128+p, f*128+j]."""
    return np.ascontiguousarray(
        wt.reshape(KD, 128, KF, 128).transpose(2, 1, 0, 3)
    )


def _chunk_w2(wt):
    """[F, D] -> [KD, 128, KF, 128]: out[d, p, k, j] = wt[k*128+p, d*128+j]."""
    return np.ascontiguousarray(
        wt.reshape(KF, 128, KD, 128).transpose(2, 1, 0, 3)
    )


last_results = None  # BassKernelResults of the most recent run (for test harness)


def kernel(x, gate_w, w1, w2, w3):
    from concourse.bass_utils import run_bass_kernel_spmd

    xf = np.ascontiguousarray(np.asarray(x, dtype=np.float32).reshape(N, D))
    gate_w = np.asarray(gate_w, dtype=np.float32)
    i1, i2, c1, c2 = _route(xf, gate_w)

    # per-expert token lists (a token appears at most once per expert)
    idxs, combs = [], []
    for e in range(E):
        a = np.where(i1 == e)[0]
        b = np.where(i2 == e)[0]
        idxs.append(np.concatenate([a, b]))
        combs.append(np.concatenate([c1[a], c2[b]]))
    loads = [len(ix) for ix in idxs]

    C0, V1, V2, flex_pieces = _plan(loads)
    Ct = C0 + V1 + V2
    if (C0, V1, V2) not in _compiled:
        _compiled[(C0, V1, V2)] = _build_bass(C0, V1, V2)
    nc = _compiled[(C0, V1, V2)]

    bf = ml_dtypes.bfloat16
    w1b = np.asarray(w1, dtype=np.float32)
    w2b = np.asarray(w2, dtype=np.float32)
    w3b = np.asarray(w3, dtype=np.float32)
    w1ce = [_chunk_w13(w1b[e].T.astype(bf)) for e in range(E)]
    w3ce = [_chunk_w13(w3b[e].T.astype(bf)) for e in range(E)]
    w2ce = [_chunk_w2(w2b[e].T.astype(bf)) for e in range(E)]

    # piece bookkeeping: per core, (expert, col_start, token_slice)
    fbase = {1: C0, 2: C0 + V1}
    cursor = [0] * E
    core_pieces = [[] for _ in range(NCORES)]
    slot_expert = [[0, 0] for _ in range(NCORES)]  # experts for slots 1, 2
    for e in range(E):
        take = min(loads[e], C0)
        core_pieces[e].append((e, 0, 0, take))
        cursor[e] = take
    for e, c, sid, cnt in flex_pieces:
        core_pieces[c].append((e, fbase[sid], cursor[e], cnt))
        slot_expert[c][sid - 1] = e
        cursor[e] += cnt

    in_maps = []
    for c in range(NCORES):
        xg = np.zeros((Ct, D), dtype=bf)
        for e, col, tok0, cnt in core_pieces[c]:
            ix = idxs[e][tok0 : tok0 + cnt]
            xg[col : col + cnt] = xf[ix].astype(bf)
        m = {
            "xt": np.ascontiguousarray(xg.T),
            "w1c": w1ce[c],
            "w3c": w3ce[c],
            "w2c": w2ce[c],
        }
        if V1:
            e1 = slot_expert[c][0]
            m["f1_w1c"], m["f1_w3c"], m["f1_w2c"] = w1ce[e1], w3ce[e1], w2ce[e1]
        if V2:
            e2 = slot_expert[c][1]
            m["f2_w1c"], m["f2_w3c"], m["f2_w2c"] = w1ce[e2], w3ce[e2], w2ce[e2]
        in_maps.append(m)

    trace = os.environ.get("BASS_KERNEL_TRACE", "") not in ("", "0")
    res = run_bass_kernel_spmd(
        nc, in_maps, core_ids=list(range(NCORES)), trace=trace
    )
    global last_results
    last_results = res

    out = np.zeros((N, D), dtype=np.float32)
    for c in range(NCORES):
        yT = np.asarray(res.results[c]["yt"], dtype=np.float32)  # [D, Ct]
        for e, col, tok0, cnt in core_pieces[c]:
            ix = idxs[e][tok0 : tok0 + cnt]
            cb = combs[e][tok0 : tok0 + cnt]
            out[ix] += cb[:, None] * yT.T[col : col + cnt]
    return out.reshape(B, T, D)


# revision 39
# speedup vs baseline: 1.0014x; 1.0012x over previous
"""MoE layer (8 experts, top-2) for 8 Trainium2 NeuronCores.

Strategy: expert-parallel with load rebalancing. Host computes the gate
(softmax + top-2) in numpy — this *is* the sharding decision — then
gathers each expert's tokens. Core e runs expert e's FFN
    y = (silu(x @ w1.T) * (x @ w3.T)) @ w2.T
on its tokens in bf16 (fp32 PSUM accumulation). Host combines with the
top-2 probabilities (scatter-add).

Load rebalancing: a uniform per-core column layout
    [ main C0 | flex1 V1 | flex2 V2 ]     (Ct = C0+V1+V2 columns)
lets overloaded experts spill overflow pieces into other cores' flex
slots. Main weights stay resident in SBUF; each flex slot's weight set
is a per-core input streamed chunk-by-chunk, with flex compute quanta
interleaved between main token tiles so the stream hides under main
compute. A greedy packer picks (C0, V1, V2) at the 16-column
granularity floor of max-core load; single-expert mode (V1=V2=0,
C0 = padded max load) is the fallback.

Device layout is feature-major ("transposed") throughout so no on-device
transposes are needed:
  xt   [D, Ct]            bf16  tokens for this core, d-major
  w1c  [KF, 128, KD, 128] bf16  w1.T chunked by output f-tile
  w3c  [KF, 128, KD, 128] bf16  w3.T chunked by output f-tile
  w2c  [KD, 128, KF, 128] bf16  w2.T chunked by output d-tile
  yt   [D, Ct]            bf16  core output, d-major
Weight chunking makes each output-tile's weights one contiguous DMA
(2 KB/partition), so compute starts after ~0.5 MB instead of 13 MB.
"""

import os
import sys
from contextlib import ExitStack

import numpy as np

sys.path.insert(0, "/opt/trn_rl_repo")

import ml_dtypes


def _ensure_axon_hooks():
    """bass_utils imports antenv.axon_hooks when tracing is requested (even
    via a stray BASS_TRACE env var); this container's antenv lacks that
    submodule. Provide a no-op fallback so the import never crashes."""
    import types

    if "antenv.axon_hooks" in sys.modules:
        return
    try:
        from antenv import axon_hooks  # noqa: F401

        return
    except ImportError:
        pass
    mod = types.ModuleType("antenv.axon_hooks")
    _state = {"hook": None}
    mod.get_axon_ntff_profile_hook = lambda: _state["hook"]
    mod.set_axon_ntff_profile_hook = lambda h: _state.__setitem__("hook", h)
    sys.modules["antenv.axon_hooks"] = mod
    try:
        import antenv

        antenv.axon_hooks = mod
    except ImportError:
        pass


_ensure_axon_hooks()

# ---- problem constants (hardcoded; kernel.py must be self-contained) ----
B, T, D, F, E, TOP_K = 8, 2048, 1024, 2048, 8, 2
N = B * T
NCORES = 8
KD = D // 128   # 8  contraction chunks over D
KF = F // 128   # 16 contraction chunks over F

_compiled = {}


def _tok_tiles(C):
    """Token-tile widths: 512s then one remainder (multiple of 16)."""
    sizes = [512] * (C // 512)
    if C % 512:
        sizes.append(C % 512)
    return sizes


def _pad16(v):
    return -(-v // 16) * 16


def _try_pack(loads, C0, V1, V2, tries=400):
    """Cut per-expert overflow (load - C0) into pieces, one piece per flex
    slot. Greedy (largest remainder into the current slot) over shuffled
    slot orders — the first try is slots-descending; retries explore
    orders where small slots trim large remainders early, which the pure
    descending greedy misses. Returns [(expert, core, slot_id, count)]
    or None."""
    import random as _random

    base = [[l - C0, e] for e, l in enumerate(loads) if l > C0]
    slots = []
    for c in range(NCORES):
        if V1:
            slots.append((V1, c, 1))
        if V2:
            slots.append((V2, c, 2))
    if sum(r for r, _ in base) > sum(s[0] for s in slots):
        return None
    rng = _random.Random(0)
    order0 = sorted(range(len(slots)), key=lambda i: -slots[i][0])
    for t in range(tries):
        order = order0 if t == 0 else rng.sample(range(len(slots)), len(slots))
        rem = [[r, e] for r, e in base]
        pieces = []
        for i in order:
            if not rem:
                break
            rem.sort(reverse=True)
            size, c, sid = slots[i]
            take = min(rem[0][0], size)
            pieces.append((rem[0][1], c, sid, take))
            rem[0][0] -= take
            if rem[0][0] == 0:
                rem.pop(0)
        if not rem:
            return pieces
    return None


def _mmcost(v):
    """Effective ns per matmul instruction at free-dim width v: the PE
    can't beat ~25ns/instr (LDWEIGHTS floor), so narrow-slot width is
    nearly free up to ~53 columns."""
    return max(25.0, 0.4167 * v + 2.7)


def _plan(loads):
    """Pick (C0, V1, V2, pieces) minimizing estimated PE time
    (160ns/main-column + per-slot instruction cost), not just Ct.
    Fallback: single-expert mode."""
    fallback = _pad16(max(loads))
    floor_ct = _pad16(-(-sum(loads) // NCORES))
    best = None
    for Ct in range(floor_ct, min(fallback, floor_ct + 64), 16):
        for V1 in range(16, 513, 16):
            for V2 in range(0, V1 + 1, 16):
                C0 = Ct - V1 - V2
                if C0 < 2048:
                    continue
                score = 160.0 * C0 + 384.0 * (
                    _mmcost(V1) + (_mmcost(V2) if V2 else 0.0)
                )
                if best is not None and score >= best[0]:
                    continue
                pieces = _try_pack(loads, C0, V1, V2)
                if pieces is not None:
                    best = (score, C0, V1, V2, pieces)
    if best is None:
        return fallback, 0, 0, []
    return best[1], best[2], best[3], best[4]


def _build_bass(C0: int, V1: int, V2: int, act: str = "Silu"):
    """Build the SPMD Bass program for layout [C0 | V1 | V2]."""
    import concourse.bacc as bacc
    import concourse.tile as tile
    from concourse import mybir

    act_fn = getattr(mybir.ActivationFunctionType, act)

    bf16 = mybir.dt.bfloat16
    f32 = mybir.dt.float32

    Ct = C0 + V1 + V2
    flex = [s for s in ((1, V1), (2, V2)) if s[1] > 0]

    nc = bacc.Bacc(
        "TRN2", target_bir_lowering=False, debug=False, num_devices=NCORES
    )
    xt = nc.declare_dram_parameter("xt", [D, Ct], bf16, isOutput=False)
    w1c = nc.declare_dram_parameter("w1c", [KF, 128, KD, 128], bf16, isOutput=False)
    w3c = nc.declare_dram_parameter("w3c", [KF, 128, KD, 128], bf16, isOutput=False)
    w2c = nc.declare_dram_parameter("w2c", [KD, 128, KF, 128], bf16, isOutput=False)
    fw = {}
    for sid, _v in flex:
        fw[sid] = tuple(
            nc.declare_dram_parameter(f"f{sid}_{nm}", shp, bf16, isOutput=False)
            for nm, shp in (
                ("w1c", [KF, 128, KD, 128]),
                ("w3c", [KF, 128, KD, 128]),
                ("w2c", [KD, 128, KF, 128]),
            )
        )
    yt = nc.declare_dram_parameter("yt", [D, Ct], bf16, isOutput=True)

    xt_r = xt.rearrange("(k p) n -> p k n", p=128)   # [128, KD, Ct]
    yt_r = yt.rearrange("(k p) n -> p k n", p=128)   # [128, KD, Ct]

    sizes = _tok_tiles(C0)
    starts = np.cumsum([0] + sizes[:-1]).tolist()
    T_tiles = len(sizes)
    fbase = {1: C0, 2: C0 + V1}
    fwidth = {1: V1, 2: V2}

    with ExitStack() as ctx:
        tc = ctx.enter_context(tile.TileContext(nc))
        wpool = ctx.enter_context(tc.tile_pool(name="w", bufs=1))
        xpool = ctx.enter_context(tc.tile_pool(name="x", bufs=3))
        hpool = ctx.enter_context(tc.tile_pool(name="h", bufs=2))
        spool = ctx.enter_context(tc.tile_pool(name="s", bufs=3))
        opool = ctx.enter_context(tc.tile_pool(name="o", bufs=4))
        psh = ctx.enter_context(tc.tile_pool(name="psh", bufs=2, space="PSUM"))
        psy = ctx.enter_context(tc.tile_pool(name="psy", bufs=2, space="PSUM"))
        if flex:
            fxpool = ctx.enter_context(tc.tile_pool(name="fx", bufs=1))
            fhpool = ctx.enter_context(tc.tile_pool(name="fh", bufs=1))
            fwpool = ctx.enter_context(tc.tile_pool(name="fw", bufs=5))
            fw2pool = ctx.enter_context(tc.tile_pool(name="fw2", bufs=4))
        warmp = ctx.enter_context(tc.tile_pool(name="warmp", bufs=1))
        warmps = ctx.enter_context(tc.tile_pool(name="warmps", bufs=1, space="PSUM"))

        # Tuned PE warm-up: the HAM clock gate needs ~5.3µs of PE-busy
        # time at 1.2 GHz before switching to 2.4 GHz, and the first real
        # matmul can't start before its DMA lands (~14µs). 12 scratch
        # matmuls (~5.1µs at mid clock) starting as soon as the engines
        # boot (~9µs) bank the ramp so real work starts at full speed.
        # The memset runs on DVE (GpSimd boots slower).
        wsrc = warmp.tile([128, 512], bf16, name="wsrc")
        nc.vector.memset(wsrc[:], 0.0)
        wdst = warmps.tile([128, 512], f32, name="wdst")
        for _ in range(14):
            nc.tensor.matmul(wdst[:], wsrc[:, 0:128], wsrc[:], start=True, stop=True)

        # Weights resident in SBUF for the whole kernel, one tile per
        # output chunk. DMA order: first f-chunk + first x tile first so
        # the PE starts as soon as the engines boot (~9µs); the rest
        # streams behind it. No PE warm-up: the clock ramp penalty
        # (~3µs at 1.2 GHz) is cheaper than idling the PE behind a
        # warm-up chain (~7µs).
        w1f = [
            wpool.tile([128, KD, 128], bf16, tag=f"w1f{f}", name=f"w1f{f}")
            for f in range(KF)
        ]
        w3f = [
            wpool.tile([128, KD, 128], bf16, tag=f"w3f{f}", name=f"w3f{f}")
            for f in range(KF)
        ]
        w2d = [
            wpool.tile([128, KF, 128], bf16, tag=f"w2d{d}", name=f"w2d{d}")
            for d in range(KD)
        ]

        def load_x(ts, tw):
            xs = [
                xpool.tile([128, tw], bf16, tag=f"xs{k}", name=f"xs{k}")
                for k in range(KD)
            ]
            for k in range(KD):
                nc.sync.dma_start(xs[k][:], xt_r[:, k, ts])
            return lambda k: xs[k][:]

        # DMA order matters twice here. First: the f=0 weights and tile-0
        # x chunks gate the first matmul, so they're split into per-k
        # strips interleaved so the earliest-consumed bytes land first
        # across ~16 parallel queues (single-queue delivery of a 256 KB
        # chunk costs ~9µs). Second: the PE consumes w1f/w3f chunks at
        # ~74 GB/s during tile 0's h-phase, so the f=1.. weight stream
        # must queue before anything not needed until later — a late
        # chunk shows up as a PE stall plus a clock re-ramp.
        # NOTE: splitting these first chunks into per-k strips to start the
        # PE earlier was tried and is a net loss: strips break the 2 KB/
        # partition contiguity of the chunk layout into 256 B bursts, DMA
        # delivery slows, and the PE stutters through f=1..6 with the
        # clock stuck at mid-speed. Whole chunks start the PE at ~14µs
        # with a clean ramp and a stream that stays ahead.
        nc.sync.dma_start(w1f[0][:], w1c[0])
        nc.sync.dma_start(w3f[0][:], w3c[0])
        xs0 = load_x(slice(0, sizes[0]), sizes[0])
        for f in range(1, KF):
            nc.sync.dma_start(w1f[f][:], w1c[f])
            nc.sync.dma_start(w3f[f][:], w3c[f])
        for d in range(KD):
            nc.sync.dma_start(w2d[d][:], w2c[d])

        # flex token columns: small, loaded once, resident; not needed
        # until tile 1's quanta, so they queue behind the weight stream.
        fxs, hsF = {}, {}
        for sid, v in flex:
            fxs[sid] = [
                fxpool.tile(
                    [128, v], bf16, tag=f"fx{sid}_{k}", name=f"fx{sid}_{k}"
                )
                for k in range(KD)
            ]
            for k in range(KD):
                nc.sync.dma_start(
                    fxs[sid][k][:], xt_r[:, k, fbase[sid] : fbase[sid] + v]
                )
            hsF[sid] = fhpool.tile(
                [128, KF, v], bf16, tag=f"hsF{sid}", name=f"hsF{sid}"
            )

        # ---- flex compute quanta (interleaved between main tiles) ----
        def flex_h(sid, f):
            v = fwidth[sid]
            w1t = fwpool.tile([128, KD, 128], bf16, tag="fw1")
            nc.sync.dma_start(w1t[:], fw[sid][0][f])
            w3t = fwpool.tile([128, KD, 128], bf16, tag="fw3")
            nc.sync.dma_start(w3t[:], fw[sid][1][f])
            ph1 = psh.tile([128, v], f32, tag="ph1")
            ph3 = psh.tile([128, v], f32, tag="ph3")
            for k in range(KD):
                nc.tensor.matmul(
                    ph1[:], w1t[:, k, :], fxs[sid][k][:],
                    start=(k == 0), stop=(k == KD - 1),
                )
            for k in range(KD):
                nc.tensor.matmul(
                    ph3[:], w3t[:, k, :], fxs[sid][k][:],
                    start=(k == 0), stop=(k == KD - 1),
                )
            sil = spool.tile([128, v], f32, tag="sil")
            nc.scalar.activation(sil[:], ph1[:], act_fn)
            nc.vector.tensor_mul(hsF[sid][:, f, :], sil[:], ph3[:])

        def flex_y(sid, d):
            v = fwidth[sid]
            w2t = fw2pool.tile([128, KF, 128], bf16, tag="fw2")
            nc.sync.dma_start(w2t[:], fw[sid][2][d])
            py = psy.tile([128, v], f32, tag="py")
            for f in range(KF):
                nc.tensor.matmul(
                    py[:], w2t[:, f, :], hsF[sid][:, f, :],
                    start=(f == 0), stop=(f == KF - 1),
                )
            yo = opool.tile([128, v], bf16, tag="yo")
            nc.vector.tensor_copy(yo[:], py[:])
            nc.sync.dma_start(yt_r[:, d, fbase[sid] : fbase[sid] + v], yo[:])

        # schedule: tile 0 stays quanta-free (its window feeds the main
        # weight stream); h-quanta spread over tiles 1..~T/2, y-quanta
        # over the remaining tiles. Each quantum eats 512 KB of streamed
        # weights in <1µs of PE time while the stream delivers at
        # ~1.4µs/chunk, so quanta are woven INSIDE the main f/d loops
        # (one per ~2 main chunks) — bunched at a tile boundary they
        # outrun the prefetch pool and stall the PE (plus clock re-ramp).
        hq = []
        for f in range(KF):
            for sid, _v in flex:
                hq.append(("h", sid, f))
        yq = []
        for d in range(KD):
            for sid, _v in flex:
                yq.append(("y", sid, d))
        sched = [[] for _ in range(T_tiles)]
        leftover = []
        h_lo = min(1, T_tiles - 1)
        h_hi = max(T_tiles // 2 + 1, h_lo + 1)   # h-quanta on tiles [h_lo, h_hi)
        n_h_tiles = h_hi - h_lo
        n_y_tiles = T_tiles - h_hi               # y-quanta on tiles [h_hi, T)
        if n_h_tiles <= 0 or n_h_tiles * 8 < len(hq):
            leftover += hq
        else:
            for i, q in enumerate(hq):
                sched[h_lo + (i * n_h_tiles) // len(hq)].append(q)
        if n_y_tiles <= 0 or n_y_tiles * 16 < len(yq):
            leftover += yq
        else:
            for i, q in enumerate(yq):
                sched[h_hi + (i * n_y_tiles) // len(yq)].append(q)

        def emit(q):
            if q[0] == "h":
                flex_h(q[1], q[2])
            else:
                flex_y(q[1], q[2])

        # ---- main tiles ----
        for t, (t0, tw) in enumerate(zip(starts, sizes)):
            ts = slice(t0, t0 + tw)
            xs = xs0 if t == 0 else load_x(ts, tw)
            qlist = sched[t]
            qi = 0

            hs = hpool.tile([128, KF, tw], bf16, tag="hs")
            for f in range(KF):
                ph1 = psh.tile([128, tw], f32, tag="ph1")
                ph3 = psh.tile([128, tw], f32, tag="ph3")
                for k in range(KD):
                    nc.tensor.matmul(
                        ph1[:], w1f[f][:, k, :], xs(k),
                        start=(k == 0), stop=(k == KD - 1),
                    )
                for k in range(KD):
                    nc.tensor.matmul(
                        ph3[:], w3f[f][:, k, :], xs(k),
                        start=(k == 0), stop=(k == KD - 1),
                    )
                sil = spool.tile([128, tw], f32, tag="sil")
                nc.scalar.activation(sil[:], ph1[:], act_fn)
                nc.vector.tensor_mul(hs[:, f, :], sil[:], ph3[:])
                if f % 2 == 1 and qi < len(qlist):
                    emit(qlist[qi])
                    qi += 1

            for d in range(KD):
                py = psy.tile([128, tw], f32, tag="py")
                for f in range(KF):
                    nc.tensor.matmul(
                        py[:], w2d[d][:, f, :], hs[:, f, :],
                        start=(f == 0), stop=(f == KF - 1),
                    )
                yo = opool.tile([128, tw], bf16, tag="yo")
                nc.vector.tensor_copy(yo[:], py[:])
                nc.sync.dma_start(yt_r[:, d, ts], yo[:])
                if qi < len(qlist):
                    emit(qlist[qi])
                    qi += 1

            while qi < len(qlist):
                emit(qlist[qi])
                qi += 1

        for q in leftover:
            emit(q)

    nc.compile()
    return nc


def _route(xf: np.ndarray, gate_w: np.ndarray):
    """Numpy replica of the reference gate: softmax + top-2 + renorm."""
    logits = xf @ gate_w.T  # [N, E] f32
    m = logits.max(axis=-1, keepdims=True)
    p = np.exp(logits - m, dtype=np.float32)
    p /= p.sum(axis=-1, keepdims=True)
    i1 = np.argmax(p, axis=-1)
    ar = np.arange(N)
    pm = p.copy()
    pm[ar, i1] = -1.0
    i2 = np.argmax(pm, axis=-1)
    p1 = p[ar, i1]
    p2 = p[ar, i2]
    s = p1 + p2
    return i1, i2, (p1 / s).astype(np.float32), (p2 / s).astype(np.float32)


def _chunk_w13(wt):
    """[D, F] -> [KF, 128, KD, 128]: out[f, p, k, j] = wt[k*128+p, f*128+j]."""
    return np.ascontiguousarray(
        wt.reshape(KD, 128, KF, 128).transpose(2, 1, 0, 3)
    )


def _chunk_w2(wt):
    """[F, D] -> [KD, 128, KF, 128]: out[d, p, k, j] = wt[k*128+p, d*128+j]."""
    return np.ascontiguousarray(
        wt.reshape(KF, 128, KD, 128).transpose(2, 1, 0, 3)
    )


last_results = None  # BassKernelResults of the most recent run (for test harness)


def kernel(x, gate_w, w1, w2, w3):
    from concourse.bass_utils import run_bass_kernel_spmd

    xf = np.ascontiguousarray(np.asarray(x, dtype=np.float32).reshape(N, D))
    gate_w = np.asarray(gate_w, dtype=np.float32)
    i1, i2, c1, c2 = _route(xf, gate_w)

    # per-expert token lists (a token appears at most once per expert)
    idxs, combs = [], []
    for e in range(E):
        a = np.where(i1 == e)[0]
        b = np.where(i2 == e)[0]
        idxs.append(np.concatenate([a, b]))
        combs.append(np.concatenate([c1[a], c2[b]]))
    loads = [len(ix) for ix in idxs]

    C0, V1, V2, flex_pieces = _plan(loads)
    Ct = C0 + V1 + V2
    if (C0, V1, V2) not in _compiled:
        _compiled[(C0, V1, V2)] = _build_bass(C0, V1, V2)
    nc = _compiled[(C0, V1, V2)]

    bf = ml_dtypes.bfloat16
    w1b = np.asarray(w1, dtype=np.float32)
    w2b = np.asarray(w2, dtype=np.float32)
    w3b = np.asarray(w3, dtype=np.float32)
    w1ce = [_chunk_w13(w1b[e].T.astype(bf)) for e in range(E)]
    w3ce = [_chunk_w13(w3b[e].T.astype(bf)) for e in range(E)]
    w2ce = [_chunk_w2(w2b[e].T.astype(bf)) for e in range(E)]

    # piece bookkeeping: per core, (expert, col_start, token_slice)
    fbase = {1: C0, 2: C0 + V1}
    cursor = [0] * E
    core_pieces = [[] for _ in range(NCORES)]
    slot_expert = [[0, 0] for _ in range(NCORES)]  # experts for slots 1, 2
    for e in range(E):
        take = min(loads[e], C0)
        core_pieces[e].append((e, 0, 0, take))
        cursor[e] = take
    for e, c, sid, cnt in flex_pieces:
        core_pieces[c].append((e, fbase[sid], cursor[e], cnt))
        slot_expert[c][sid - 1] = e
        cursor[e] += cnt

    in_maps = []
    for c in range(NCORES):
        xg = np.zeros((Ct, D), dtype=bf)
        for e, col, tok0, cnt in core_pieces[c]:
            ix = idxs[e][tok0 : tok0 + cnt]
            xg[col : col + cnt] = xf[ix].astype(bf)
        m = {
            "xt": np.ascontiguousarray(xg.T),
            "w1c": w1ce[c],
            "w3c": w3ce[c],
            "w2c": w2ce[c],
        }
        if V1:
            e1 = slot_expert[c][0]
            m["f1_w1c"], m["f1_w3c"], m["f1_w2c"] = w1ce[e1], w3ce[e1], w2ce[e1]
        if V2:
            e2 = slot_expert[c][1]
            m["f2_w1c"], m["f2_w3c"], m["f2_w2c"] = w1ce[e2], w3ce[e2], w2ce[e2]
        in_maps.append(m)

    trace = os.environ.get("BASS_KERNEL_TRACE", "") not in ("", "0")
    res = run_bass_kernel_spmd(
        nc, in_maps, core_ids=list(range(NCORES)), trace=trace
    )
    global last_results
    last_results = res

    out = np.zeros((N, D), dtype=np.float32)
    for c in range(NCORES):
        yT = np.asarray(res.results[c]["yt"], dtype=np.float32)  # [D, Ct]
        for e, col, tok0, cnt in core_pieces[c]:
            ix = idxs[e][tok0 : tok0 + cnt]
            cb = combs[e][tok0 : tok0 + cnt]
            out[ix] += cb[:, None] * yT.T[col : col + cnt]
    return out.reshape(B, T, D)
